# revision 7
# baseline (speedup 1.0000x reference)
"""Trainium2 Bass kernel for InputProjection + time/sensor masking + LayerNorm.

Reference computation (B=64, T=4096, C=51, D=64):
    mask[b,t,c] = time_mask[b,t] | sensor_mask[b,c]
    out = LN( einsum('btc,cd->btd', x*(1-mask), W) + einsum('btc,cd->btd', mask, Wm) )

Algebraic restructure (exact):
    With W_b[c,d]   = (1 - sm[b,c]) * W[c,d]
         smWm_b[d]  = sum_c sm[b,c]*Wm[c,d]
         allWm[d]   = sum_c Wm[c,d]
    pre[b,t,d] = sum_c x[b,t,c]*(1-tm[b,t]) * W_b[c,d]
               + 1 * smWm_b[d]
               + tm[b,t] * (allWm - smWm_b)[d]
    (for tm=1 rows the x-term vanishes and pre = allWm exactly, whose LN equals the
     reference's masked-row output; no select needed.)

Device kernel v2 (per core, data-parallel over batch; all I/O bf16):
    - augmented transposed inputs, two batches packed per 128 partitions:
        xaug[pair, half, 53, nj, 128]: rows 0..50 = (x*(1-tm)).T, row 51 = 1,
            row 52 = tm; chunk j holds tokens t = m*nj + j (m = psum partition)
            and is contiguous [53,128] for fast weight load.
        waug[pair, half, 53, D]: rows 0..50 = W_b, 51 = smWm_b, 52 = allWm-smWm_b.
    - per 128-token chunk: one 53-deep bf16 matmul -> PSUM [128t, 64d] fp32
    - per 2 PSUM banks (16 chunks): one wide ACT Copy evict -> SBUF bf16
      (amortizes the ~352-cycle ACT fixed overhead)
    - per chunk pair: one bn_stats whose INPUT AP interleaves the two chunks
      element-wise ([128, 64d, 2]) so the even/odd triple split lands on
      chunk A/B exactly -- full per-chunk stats, no combine chain, and the
      matmul PSUM writes stay contiguous. (The BIR verifier requires 6
      stats/partition per bn_stats, so multi-group 3D bn_stats is out.)
    - per pair: short s/b chain (rsqrt via ACT Sqrt + DVE reciprocal;
      tolerance 2e-2 so no Newton step)
    - per chunk: in-place DVE tensor_scalar (mult,add) apply on the bf16 SBUF
      copy (4x perf mode) then DMA out (each partition writes one contiguous
      nj*64*2B block).
    gamma/beta applied on host only if nontrivial (reference uses 1/0).
"""

import os
import sys
from contextlib import ExitStack

import numpy as np
import ml_dtypes

for _p in ("/opt/trn_rl_repo", "/root/.axon_site/_ro/trn_rl_repo"):
    if os.path.isdir(_p) and _p not in sys.path:
        sys.path.insert(0, _p)

import concourse.bass as bass
import concourse.bacc as bacc
import concourse.mybir as mybir
from concourse import tile
from concourse.bass_utils import run_bass_kernel_spmd

F32 = mybir.dt.float32
BF16 = mybir.dt.bfloat16
AF = mybir.ActivationFunctionType
ALU = mybir.AluOpType

B, T, C, D = 64, 4096, 51, 64
LN_EPS = 1e-5
N_CORES = 8
BPC = B // N_CORES          # batches per core
NPAIR = BPC // 2            # batch pairs per core
CAUG = C + 2                # augmented contraction depth (x rows + ones + tm)
MTILE = 128                 # tokens per matmul chunk (psum partitions)
BANK = 8                    # chunks per PSUM bank (8*64 fp32 = 512 = one bank)


def _bn_stats_stream(nc, out_ap, in_ap):
    """bn_stats with a multi-dim input AP treated as ONE positional stream.

    The HW's even/odd triple split is by stream position (dual accumulator
    pipes), so a [128, d, 2] interleaving AP yields chunk-A stats in the even
    triple and chunk-B in the odd one. bass's bn_stats wrapper would treat the
    extra AP dim as a stats "group" and demand a 6*G output (which the BIR
    verifier rejects anyway); emit the raw instruction instead.
    """
    eng = nc.vector
    return eng.add_instruction(
        mybir.InstBNStats(
            name=eng.bass.get_next_instruction_name(),
            ins=[eng.lower_ap(in_ap)],
            outs=[eng.lower_ap(out_ap)],
        )
    )


def build_nc(npair: int, t_len: int, debug: bool = False):
    """Build the per-core Bass program. Identical on all cores (SPMD)."""
    nj = t_len // MTILE                 # chunks per batch
    assert t_len % (MTILE * BANK) == 0, "t_len must be a multiple of 1024"
    nbank = nj // BANK                  # psum banks per batch

    nc = bacc.Bacc("TRN2", target_bir_lowering=False, debug=debug)
    xaug_d = nc.dram_tensor("xaug", [npair, 2, CAUG, nj, MTILE], BF16,
                            kind="ExternalInput")
    waug_d = nc.dram_tensor("waug", [npair, 2, CAUG, D], BF16,
                            kind="ExternalInput")
    out_d = nc.dram_tensor("out", [2 * npair, t_len, D], BF16,
                           kind="ExternalOutput")

    with tile.TileContext(nc) as tc, ExitStack() as ctx:
        wpool = ctx.enter_context(tc.tile_pool(name="wpool", bufs=1))
        xpool = ctx.enter_context(tc.tile_pool(name="xpool", bufs=3))
        opool = ctx.enter_context(tc.tile_pool(name="opool", bufs=3))
        spool = ctx.enter_context(tc.tile_pool(name="spool", bufs=2))
        tpool = ctx.enter_context(tc.tile_pool(name="tpool", bufs=2))
        psum = ctx.enter_context(tc.tile_pool(name="psum", bufs=4, space="PSUM"))

        wa = wpool.tile([128, npair, D], BF16)
        nc.sync.dma_start(wa[0:CAUG], waug_d[:, 0].rearrange("n k d -> k n d"))
        nc.sync.dma_start(wa[64:64 + CAUG],
                          waug_d[:, 1].rearrange("n k d -> k n d"))

        for p in range(npair):
            xa = xpool.tile([128, nj, MTILE], BF16)
            nc.sync.dma_start(xa[0:CAUG], xaug_d[p, 0])
            nc.sync.dma_start(xa[64:64 + CAUG], xaug_d[p, 1])

            # stats triples per chunk pair: slots 0..2 = even chunk (2q),
            # slots 3..5 = odd chunk (2q+1); count = 64 each.
            stp = spool.tile([128, 2, nj // 2, 6], F32)
            obs = []
            for i in range(2):
                rb = 64 * i
                ob = opool.tile([128, nj, D], BF16, tag="ob")
                obs.append(ob)
                for h in range(nbank // 2):
                    ps = psum.tile([128, 2, BANK, D], F32, tag="psbank")
                    for hb in range(2):
                        for q in range(BANK):
                            j = (2 * h + hb) * BANK + q
                            nc.tensor.matmul(
                                ps[:, hb, q, :],
                                xa[rb:rb + CAUG, j, :],
                                wa[rb:rb + CAUG, p, :],
                                start=True,
                                stop=True,
                            )
                        # interleave-AP bn_stats per chunk pair: stream
                        # A0,B0,A1,B1,... so even/odd triples = chunk A/B
                        for q in range(BANK // 2):
                            g = (2 * h + hb) * (BANK // 2) + q
                            _bn_stats_stream(
                                nc, stp[:, i, g, :],
                                ps[:, hb, 2 * q:2 * q + 2, :].rearrange(
                                    "p a d -> p d a"))
                    # wide evict: two banks -> bf16 SBUF in one ACT op
                    nc.scalar.activation(
                        ob[:, 2 * h * BANK:2 * (h + 1) * BANK, :], ps[:],
                        AF.Copy)

            # s = rsqrt(var+eps), b = -mu*s per chunk; run the short chain
            # once over the even-chunk slots and once over the odd-chunk
            # slots ([128, 2*nj/2] strided views, count=64 -> var = M2/64).
            sab = []
            for mf, cf in ((1, 2), (4, 5)):
                mu = stp[:, :, :, mf]
                M2 = stp[:, :, :, cf]
                veps = tpool.tile([128, 2, nj // 2], F32, tag=f"veps{mf}")
                sq = tpool.tile([128, 2, nj // 2], F32, tag=f"sq{mf}")
                rr = tpool.tile([128, 2, nj // 2], F32, tag=f"rr{mf}")
                bb = tpool.tile([128, 2, nj // 2], F32, tag=f"bb{mf}")
                nc.vector.tensor_scalar(veps[:], M2, 1.0 / D, LN_EPS,
                                        ALU.mult, ALU.add)
                nc.scalar.activation(sq[:], veps[:], AF.Sqrt)
                nc.vector.reciprocal(rr[:], sq[:])
                nc.vector.scalar_tensor_tensor(
                    bb[:], mu, -1.0, rr[:], ALU.mult, ALU.mult)
                sab.append((rr, bb))

            for i in range(2):
                ob = obs[i]
                for j in range(nj):
                    rr, bb = sab[j % 2]
                    g = j // 2
                    nc.vector.tensor_scalar(
                        ob[:, j, :], ob[:, j, :],
                        rr[:, i, g:g + 1], bb[:, i, g:g + 1],
                        ALU.mult, ALU.add)
                nc.sync.dma_start(
                    out_d[2 * p + i].rearrange("(k j) d -> k j d", k=128),
                    ob[:])
    nc.compile()
    return nc


def _host_prep(x, W, Wm, time_mask, sensor_mask, n_cores):
    """Shard along batch; transpose/augment per-core inputs (bf16)."""
    b, t_len, c = x.shape
    d = W.shape[1]
    bpc = b // n_cores
    npair = bpc // 2
    nj = t_len // MTILE

    tm = np.ascontiguousarray(time_mask).astype(np.float32)
    sm = np.ascontiguousarray(sensor_mask).astype(np.float32)
    x = np.asarray(x, dtype=np.float32)
    W = np.asarray(W, dtype=np.float32)
    Wm = np.asarray(Wm, dtype=np.float32)

    xm = x * (1.0 - tm)[:, :, None]
    xaug = np.empty((b, CAUG, t_len), np.float32)
    xaug[:, :c] = xm.transpose(0, 2, 1)
    xaug[:, c] = 1.0
    xaug[:, c + 1] = tm
    # token t = m*nj + j -> chunk j contiguous [CAUG, 128]
    xaug = xaug.reshape(b, CAUG, MTILE, nj).transpose(0, 1, 3, 2)
    xaug = xaug.astype(ml_dtypes.bfloat16)

    allWm = Wm.sum(axis=0)
    smWm = sm @ Wm
    waug = np.empty((b, CAUG, d), np.float32)
    waug[:, :c] = W[None] * (1.0 - sm)[:, :, None]
    waug[:, c] = smWm
    waug[:, c + 1] = allWm[None] - smWm
    waug = waug.astype(ml_dtypes.bfloat16)

    in_maps = []
    for m in range(n_cores):
        sl = slice(m * bpc, (m + 1) * bpc)
        in_maps.append({
            "xaug": np.ascontiguousarray(
                xaug[sl].reshape(npair, 2, CAUG, nj, MTILE)),
            "waug": np.ascontiguousarray(
                waug[sl].reshape(npair, 2, CAUG, d)),
        })
    return in_maps


_NC_CACHE = {}


def kernel(x, W, Wm, gamma, beta, time_mask, sensor_mask):
    x = np.asarray(x)
    b, t_len, c = x.shape
    n_cores = N_CORES
    bpc = b // n_cores
    npair = bpc // 2

    key = (npair, t_len)
    if key not in _NC_CACHE:
        _NC_CACHE[key] = build_nc(npair, t_len)
    nc = _NC_CACHE[key]

    in_maps = _host_prep(x, W, Wm, time_mask, sensor_mask, n_cores)

    trace = bool(int(os.environ.get("KERNEL_TRACE", "0")))
    res = run_bass_kernel_spmd(nc, in_maps, list(range(n_cores)), trace=trace)
    kernel.last_results = res

    out = np.concatenate(
        [np.asarray(res.results[i]["out"]) for i in range(n_cores)], axis=0)

    out = out.astype(np.float32)
    gamma = np.asarray(gamma, dtype=np.float32)
    beta = np.asarray(beta, dtype=np.float32)
    if not (np.all(gamma == 1.0) and np.all(beta == 0.0)):
        out = out * gamma + beta
    return out


# revision 15
# speedup vs baseline: 1.7992x; 1.7992x over previous
"""Trainium2 Bass kernel for InputProjection + time/sensor masking + LayerNorm.

Reference computation (B=64, T=4096, C=51, D=64):
    mask[b,t,c] = time_mask[b,t] | sensor_mask[b,c]
    out = LN( einsum('btc,cd->btd', x*(1-mask), W) + einsum('btc,cd->btd', mask, Wm) )

Algebraic restructure (exact):
    With W_b[c,d]   = (1 - sm[b,c]) * W[c,d]
         smWm_b[d]  = sum_c sm[b,c]*Wm[c,d]
         allWm[d]   = sum_c Wm[c,d]
    pre[b,t,d] = sum_c x[b,t,c]*(1-tm[b,t]) * W_b[c,d]
               + 1 * smWm_b[d]
               + tm[b,t] * (allWm - smWm_b)[d]
    (for tm=1 rows the x-term vanishes and pre = allWm exactly, whose LN equals the
     reference's masked-row output; no select needed.)

Device kernel v2 (per core, data-parallel over batch; all I/O bf16):
    - augmented transposed inputs, two batches packed per 128 partitions:
        xaug[pair, half, 53, nj, 128]: rows 0..50 = (x*(1-tm)).T, row 51 = 1,
            row 52 = tm; chunk j holds tokens t = m*nj + j (m = psum partition)
            and is contiguous [53,128] for fast weight load.
        waug[pair, half, 53, D]: rows 0..50 = W_b, 51 = smWm_b, 52 = allWm-smWm_b.
    - per 128-token chunk: one 53-deep bf16 matmul -> PSUM [128t, 64d] fp32
    - per 2 PSUM banks (16 chunks): one wide ACT Copy evict -> SBUF bf16
      (amortizes the ~352-cycle ACT fixed overhead)
    - per chunk pair: one bn_stats whose INPUT AP interleaves the two chunks
      element-wise ([128, 64d, 2]) so the even/odd triple split lands on
      chunk A/B exactly -- full per-chunk stats, no combine chain, and the
      matmul PSUM writes stay contiguous. (The BIR verifier requires 6
      stats/partition per bn_stats, so multi-group 3D bn_stats is out.)
    - per pair: short s/b chain (rsqrt via ACT Sqrt + DVE reciprocal;
      tolerance 2e-2 so no Newton step)
    - per chunk: in-place DVE tensor_scalar (mult,add) apply on the bf16 SBUF
      copy (4x perf mode) then DMA out (each partition writes one contiguous
      nj*64*2B block).
    gamma/beta applied on host only if nontrivial (reference uses 1/0).
"""

import os
import sys
from contextlib import ExitStack

import numpy as np
import ml_dtypes

for _p in ("/opt/trn_rl_repo", "/root/.axon_site/_ro/trn_rl_repo"):
    if os.path.isdir(_p) and _p not in sys.path:
        sys.path.insert(0, _p)

import concourse.bass as bass
import concourse.bacc as bacc
import concourse.mybir as mybir
from concourse import tile
from concourse.bass_utils import run_bass_kernel_spmd

F32 = mybir.dt.float32
BF16 = mybir.dt.bfloat16
AF = mybir.ActivationFunctionType
ALU = mybir.AluOpType

B, T, C, D = 64, 4096, 51, 64
LN_EPS = 1e-5
N_CORES = 8
BPC = B // N_CORES          # batches per core
NPAIR = BPC // 2            # batch pairs per core
CAUG = C + 2                # augmented contraction depth (x rows + ones + tm)
MTILE = 128                 # tokens per matmul chunk (psum partitions)
BANK = 8                    # chunks per PSUM bank (8*64 fp32 = 512 = one bank)


def _bn_stats_stream(nc, out_ap, in_ap):
    """bn_stats with a multi-dim input AP treated as ONE positional stream.

    The HW's even/odd triple split is by stream position (dual accumulator
    pipes), so a [128, d, 2] interleaving AP yields chunk-A stats in the even
    triple and chunk-B in the odd one. bass's bn_stats wrapper would treat the
    extra AP dim as a stats "group" and demand a 6*G output (which the BIR
    verifier rejects anyway); emit the raw instruction instead.
    """
    eng = nc.vector
    return eng.add_instruction(
        mybir.InstBNStats(
            name=eng.bass.get_next_instruction_name(),
            ins=[eng.lower_ap(in_ap)],
            outs=[eng.lower_ap(out_ap)],
        )
    )


def build_nc(npair: int, t_len: int, debug: bool = False):
    """Build the per-core Bass program. Identical on all cores (SPMD)."""
    nj = t_len // MTILE                 # chunks per batch
    assert t_len % (MTILE * BANK) == 0, "t_len must be a multiple of 1024"
    nbank = nj // BANK                  # psum banks per batch

    nc = bacc.Bacc("TRN2", target_bir_lowering=False, debug=debug)
    # full 128-partition, flat-2D DMA shapes: partial-partition / 3D-AP
    # transfers fall off the distributed DGE path onto a single serialized
    # queue (measured: 930 descriptors on DMA_0 + 5.5us DIRECT2D per
    # dma_start on Sync). The 22 zero partitions cost ~20% extra bytes but
    # keep all 16 DMA engines fed.
    xaug_d = nc.dram_tensor("xaug", [npair, 128, t_len], BF16,
                            kind="ExternalInput")
    waug_d = nc.dram_tensor("waug", [npair, 128, D], BF16,
                            kind="ExternalInput")
    out_d = nc.dram_tensor("out", [2 * npair, t_len, D], BF16,
                           kind="ExternalOutput")

    with tile.TileContext(nc) as tc, ExitStack() as ctx:
        wpool = ctx.enter_context(tc.tile_pool(name="wpool", bufs=1))
        xpool = ctx.enter_context(tc.tile_pool(name="xpool", bufs=3))
        opool = ctx.enter_context(tc.tile_pool(name="opool", bufs=3))
        spool = ctx.enter_context(tc.tile_pool(name="spool", bufs=2))
        tpool = ctx.enter_context(tc.tile_pool(name="tpool", bufs=2))
        psum = ctx.enter_context(tc.tile_pool(name="psum", bufs=4, space="PSUM"))

        wa = wpool.tile([128, npair, D], BF16)
        nc.sync.dma_start(wa[:], waug_d.rearrange("n k d -> k n d"))

        for p in range(npair):
            xat = xpool.tile([128, t_len], BF16)
            nc.sync.dma_start(xat[:], xaug_d[p])
            # chunk j = contiguous 128-col block (token t = m*nj + j)
            xa = xat[:].rearrange("k (j m) -> k j m", m=MTILE)

            # stats triples per chunk pair: slots 0..2 = even chunk (2q),
            # slots 3..5 = odd chunk (2q+1); count = 64 each.
            stp = spool.tile([128, 2, nj // 2, 6], F32)
            obs = []
            for i in range(2):
                rb = 64 * i
                ob = opool.tile([128, nj, D], BF16, tag="ob")
                obs.append(ob)
                for h in range(nbank // 2):
                    ps = psum.tile([128, 2, BANK, D], F32, tag="psbank")
                    for hb in range(2):
                        for q in range(BANK):
                            j = (2 * h + hb) * BANK + q
                            nc.tensor.matmul(
                                ps[:, hb, q, :],
                                xa[rb:rb + CAUG, j, :],
                                wa[rb:rb + CAUG, p, :],
                                start=True,
                                stop=True,
                            )
                        # interleave-AP bn_stats per chunk pair: stream
                        # A0,B0,A1,B1,... so even/odd triples = chunk A/B
                        for q in range(BANK // 2):
                            g = (2 * h + hb) * (BANK // 2) + q
                            _bn_stats_stream(
                                nc, stp[:, i, g, :],
                                ps[:, hb, 2 * q:2 * q + 2, :].rearrange(
                                    "p a d -> p d a"))
                    # wide evict: two banks -> bf16 SBUF in one ACT op
                    nc.scalar.activation(
                        ob[:, 2 * h * BANK:2 * (h + 1) * BANK, :], ps[:],
                        AF.Copy)

            # s = rsqrt(var+eps), b = -mu*s per chunk; run the short chain
            # once over the even-chunk slots and once over the odd-chunk
            # slots ([128, 2*nj/2] strided views, count=64 -> var = M2/64).
            sab = []
            for mf, cf in ((1, 2), (4, 5)):
                mu = stp[:, :, :, mf]
                M2 = stp[:, :, :, cf]
                veps = tpool.tile([128, 2, nj // 2], F32, tag=f"veps{mf}")
                sq = tpool.tile([128, 2, nj // 2], F32, tag=f"sq{mf}")
                rr = tpool.tile([128, 2, nj // 2], F32, tag=f"rr{mf}")
                bb = tpool.tile([128, 2, nj // 2], F32, tag=f"bb{mf}")
                nc.vector.tensor_scalar(veps[:], M2, 1.0 / D, LN_EPS,
                                        ALU.mult, ALU.add)
                nc.scalar.activation(sq[:], veps[:], AF.Sqrt)
                nc.vector.reciprocal(rr[:], sq[:])
                nc.vector.scalar_tensor_tensor(
                    bb[:], mu, -1.0, rr[:], ALU.mult, ALU.mult)
                sab.append((rr, bb))

            for i in range(2):
                ob = obs[i]
                for j in range(nj):
                    rr, bb = sab[j % 2]
                    g = j // 2
                    nc.vector.tensor_scalar(
                        ob[:, j, :], ob[:, j, :],
                        rr[:, i, g:g + 1], bb[:, i, g:g + 1],
                        ALU.mult, ALU.add)
                nc.sync.dma_start(
                    out_d[2 * p + i].rearrange("(k j) d -> k j d", k=128),
                    ob[:])
    nc.compile()
    return nc


def _host_prep(x, W, Wm, time_mask, sensor_mask, n_cores):
    """Shard along batch; transpose/augment per-core inputs (bf16)."""
    b, t_len, c = x.shape
    d = W.shape[1]
    bpc = b // n_cores
    npair = bpc // 2
    nj = t_len // MTILE

    tm = np.ascontiguousarray(time_mask).astype(np.float32)
    sm = np.ascontiguousarray(sensor_mask).astype(np.float32)
    x = np.asarray(x, dtype=np.float32)
    W = np.asarray(W, dtype=np.float32)
    Wm = np.asarray(Wm, dtype=np.float32)

    xm = x * (1.0 - tm)[:, :, None]
    # pair-packed 128 partitions: batch A rows 0..52, batch B rows 64..116
    xaug = np.zeros((b // 2, 128, t_len), np.float32)
    xpairs = xm.reshape(b // 2, 2, t_len, c)
    tmp = tm.reshape(b // 2, 2, t_len)
    for half in range(2):
        rb = 64 * half
        xaug[:, rb:rb + c] = xpairs[:, half].transpose(0, 2, 1)
        xaug[:, rb + c] = 1.0
        xaug[:, rb + c + 1] = tmp[:, half]
    # free layout (j, m): token t = m*nj + j -> chunk j contiguous [*, 128]
    nj = t_len // MTILE
    xaug = (xaug.reshape(b // 2, 128, MTILE, nj).transpose(0, 1, 3, 2)
            .reshape(b // 2, 128, t_len))
    xaug = xaug.astype(ml_dtypes.bfloat16)

    allWm = Wm.sum(axis=0)
    smWm = sm @ Wm
    waug_b = np.empty((b, CAUG, d), np.float32)
    waug_b[:, :c] = W[None] * (1.0 - sm)[:, :, None]
    waug_b[:, c] = smWm
    waug_b[:, c + 1] = allWm[None] - smWm
    waug = np.zeros((b // 2, 128, d), np.float32)
    wpairs = waug_b.reshape(b // 2, 2, CAUG, d)
    waug[:, 0:CAUG] = wpairs[:, 0]
    waug[:, 64:64 + CAUG] = wpairs[:, 1]
    waug = waug.astype(ml_dtypes.bfloat16)

    in_maps = []
    for m in range(n_cores):
        sl = slice(m * npair, (m + 1) * npair)
        in_maps.append({
            "xaug": np.ascontiguousarray(xaug[sl]),
            "waug": np.ascontiguousarray(waug[sl]),
        })
    return in_maps


_NC_CACHE = {}


def kernel(x, W, Wm, gamma, beta, time_mask, sensor_mask):
    x = np.asarray(x)
    b, t_len, c = x.shape
    n_cores = N_CORES
    bpc = b // n_cores
    npair = bpc // 2

    key = (npair, t_len)
    if key not in _NC_CACHE:
        _NC_CACHE[key] = build_nc(npair, t_len)
    nc = _NC_CACHE[key]

    in_maps = _host_prep(x, W, Wm, time_mask, sensor_mask, n_cores)

    trace = bool(int(os.environ.get("KERNEL_TRACE", "0")))
    res = run_bass_kernel_spmd(nc, in_maps, list(range(n_cores)), trace=trace)
    kernel.last_results = res

    out = np.concatenate(
        [np.asarray(res.results[i]["out"]) for i in range(n_cores)], axis=0)

    out = out.astype(np.float32)
    gamma = np.asarray(gamma, dtype=np.float32)
    beta = np.asarray(beta, dtype=np.float32)
    if not (np.all(gamma == 1.0) and np.all(beta == 0.0)):
        out = out * gamma + beta
    return out


# revision 17
# speedup vs baseline: 2.4071x; 1.3379x over previous
"""Trainium2 Bass kernel for InputProjection + time/sensor masking + LayerNorm.

Reference computation (B=64, T=4096, C=51, D=64):
    mask[b,t,c] = time_mask[b,t] | sensor_mask[b,c]
    out = LN( einsum('btc,cd->btd', x*(1-mask), W) + einsum('btc,cd->btd', mask, Wm) )

Algebraic restructure (exact):
    With W_b[c,d]   = (1 - sm[b,c]) * W[c,d]
         smWm_b[d]  = sum_c sm[b,c]*Wm[c,d]
         allWm[d]   = sum_c Wm[c,d]
    pre[b,t,d] = sum_c x[b,t,c]*(1-tm[b,t]) * W_b[c,d]
               + 1 * smWm_b[d]
               + tm[b,t] * (allWm - smWm_b)[d]
    (for tm=1 rows the x-term vanishes and pre = allWm exactly, whose LN equals the
     reference's masked-row output; no select needed.)

Device kernel v2 (per core, data-parallel over batch; all I/O bf16):
    - augmented transposed inputs, two batches packed per 128 partitions:
        xaug[pair, half, 53, nj, 128]: rows 0..50 = (x*(1-tm)).T, row 51 = 1,
            row 52 = tm; chunk j holds tokens t = m*nj + j (m = psum partition)
            and is contiguous [53,128] for fast weight load.
        waug[pair, half, 53, D]: rows 0..50 = W_b, 51 = smWm_b, 52 = allWm-smWm_b.
    - per 128-token chunk: one 53-deep bf16 matmul -> PSUM [128t, 64d] fp32
    - per 2 PSUM banks (16 chunks): one wide ACT Copy evict -> SBUF bf16
      (amortizes the ~352-cycle ACT fixed overhead)
    - per chunk pair: one bn_stats whose INPUT AP interleaves the two chunks
      element-wise ([128, 64d, 2]) so the even/odd triple split lands on
      chunk A/B exactly -- full per-chunk stats, no combine chain, and the
      matmul PSUM writes stay contiguous. (The BIR verifier requires 6
      stats/partition per bn_stats, so multi-group 3D bn_stats is out.)
    - per pair: short s/b chain (rsqrt via ACT Sqrt + DVE reciprocal;
      tolerance 2e-2 so no Newton step)
    - per chunk: in-place DVE tensor_scalar (mult,add) apply on the bf16 SBUF
      copy (4x perf mode) then DMA out (each partition writes one contiguous
      nj*64*2B block).
    gamma/beta applied on host only if nontrivial (reference uses 1/0).
"""

import os
import sys
from contextlib import ExitStack

import numpy as np
import ml_dtypes

for _p in ("/opt/trn_rl_repo", "/root/.axon_site/_ro/trn_rl_repo"):
    if os.path.isdir(_p) and _p not in sys.path:
        sys.path.insert(0, _p)

import concourse.bass as bass
import concourse.bacc as bacc
import concourse.mybir as mybir
from concourse import tile
from concourse.bass_utils import run_bass_kernel_spmd

F32 = mybir.dt.float32
BF16 = mybir.dt.bfloat16
AF = mybir.ActivationFunctionType
ALU = mybir.AluOpType

B, T, C, D = 64, 4096, 51, 64
LN_EPS = 1e-5
N_CORES = 8
BPC = B // N_CORES          # batches per core
NPAIR = BPC // 2            # batch pairs per core
CAUG = C + 2                # augmented contraction depth (x rows + ones + tm)
MTILE = 128                 # tokens per matmul chunk (psum partitions)
BANK = 8                    # chunks per PSUM bank (8*64 fp32 = 512 = one bank)
# per-chunk LN-apply engine routing (V=DVE ~203ns, A=ACT ~347ns, G=GPSIMD ?):
# DVE also carries bn_stats (27us), ACT the wide evicts (22us), GPSIMD is idle
APPLY_ROUTE = tuple(os.environ.get("KERNEL_APPLY_ROUTE", "VGAVGAVG"))


def _bn_stats_stream(nc, out_ap, in_ap):
    """bn_stats with a multi-dim input AP treated as ONE positional stream.

    The HW's even/odd triple split is by stream position (dual accumulator
    pipes), so a [128, d, 2] interleaving AP yields chunk-A stats in the even
    triple and chunk-B in the odd one. bass's bn_stats wrapper would treat the
    extra AP dim as a stats "group" and demand a 6*G output (which the BIR
    verifier rejects anyway); emit the raw instruction instead.
    """
    eng = nc.vector
    return eng.add_instruction(
        mybir.InstBNStats(
            name=eng.bass.get_next_instruction_name(),
            ins=[eng.lower_ap(in_ap)],
            outs=[eng.lower_ap(out_ap)],
        )
    )


def build_nc(npair: int, t_len: int, debug: bool = False):
    """Build the per-core Bass program. Identical on all cores (SPMD)."""
    nj = t_len // MTILE                 # chunks per batch
    assert t_len % (MTILE * BANK) == 0, "t_len must be a multiple of 1024"
    nbank = nj // BANK                  # psum banks per batch

    nc = bacc.Bacc("TRN2", target_bir_lowering=False, debug=debug)
    # full 128-partition, flat-2D DMA shapes: partial-partition / 3D-AP
    # transfers fall off the distributed DGE path onto a single serialized
    # queue (measured: 930 descriptors on DMA_0 + 5.5us DIRECT2D per
    # dma_start on Sync). The 22 zero partitions cost ~20% extra bytes but
    # keep all 16 DMA engines fed.
    xaug_d = nc.dram_tensor("xaug", [npair, 128, t_len], BF16,
                            kind="ExternalInput")
    waug_d = nc.dram_tensor("waug", [npair, 128, D], BF16,
                            kind="ExternalInput")
    out_d = nc.dram_tensor("out", [2 * npair, t_len, D], BF16,
                           kind="ExternalOutput")

    with tile.TileContext(nc) as tc, ExitStack() as ctx:
        wpool = ctx.enter_context(tc.tile_pool(name="wpool", bufs=1))
        xpool = ctx.enter_context(tc.tile_pool(name="xpool", bufs=3))
        opool = ctx.enter_context(tc.tile_pool(name="opool", bufs=3))
        spool = ctx.enter_context(tc.tile_pool(name="spool", bufs=2))
        tpool = ctx.enter_context(tc.tile_pool(name="tpool", bufs=2))
        psum = ctx.enter_context(tc.tile_pool(name="psum", bufs=4, space="PSUM"))

        wa = wpool.tile([128, npair, D], BF16)
        nc.sync.dma_start(wa[:], waug_d.rearrange("n k d -> k n d"))

        for p in range(npair):
            xat = xpool.tile([128, t_len], BF16)
            nc.sync.dma_start(xat[:], xaug_d[p])
            # chunk j = contiguous 128-col block (token t = m*nj + j)
            xa = xat[:].rearrange("k (j m) -> k j m", m=MTILE)

            # stats triples per chunk pair: slots 0..2 = even chunk (2q),
            # slots 3..5 = odd chunk (2q+1); count = 64 each.
            stp = spool.tile([128, 2, nj // 2, 6], F32)
            obs = []
            for i in range(2):
                rb = 64 * i
                ob = opool.tile([128, nj, D], BF16, tag="ob")
                obs.append(ob)
                for h in range(nbank // 2):
                    ps = psum.tile([128, 2, BANK, D], F32, tag="psbank")
                    for hb in range(2):
                        for q in range(BANK):
                            j = (2 * h + hb) * BANK + q
                            nc.tensor.matmul(
                                ps[:, hb, q, :],
                                xa[rb:rb + CAUG, j, :],
                                wa[rb:rb + CAUG, p, :],
                                start=True,
                                stop=True,
                            )
                        # interleave-AP bn_stats per chunk pair: stream
                        # A0,B0,A1,B1,... so even/odd triples = chunk A/B
                        for q in range(BANK // 2):
                            g = (2 * h + hb) * (BANK // 2) + q
                            _bn_stats_stream(
                                nc, stp[:, i, g, :],
                                ps[:, hb, 2 * q:2 * q + 2, :].rearrange(
                                    "p a d -> p d a"))
                    # wide evict: two banks -> bf16 SBUF in one ACT op
                    nc.scalar.activation(
                        ob[:, 2 * h * BANK:2 * (h + 1) * BANK, :], ps[:],
                        AF.Copy)

            # s = rsqrt(var+eps), b = -mu*s per chunk; run the short chain
            # once over the even-chunk slots and once over the odd-chunk
            # slots ([128, 2*nj/2] strided views, count=64 -> var = M2/64).
            sab = []
            for mf, cf in ((1, 2), (4, 5)):
                mu = stp[:, :, :, mf]
                M2 = stp[:, :, :, cf]
                veps = tpool.tile([128, 2, nj // 2], F32, tag=f"veps{mf}")
                sq = tpool.tile([128, 2, nj // 2], F32, tag=f"sq{mf}")
                rr = tpool.tile([128, 2, nj // 2], F32, tag=f"rr{mf}")
                bb = tpool.tile([128, 2, nj // 2], F32, tag=f"bb{mf}")
                nc.vector.tensor_scalar(veps[:], M2, 1.0 / D, LN_EPS,
                                        ALU.mult, ALU.add)
                nc.scalar.activation(sq[:], veps[:], AF.Sqrt)
                nc.vector.reciprocal(rr[:], sq[:])
                nc.vector.scalar_tensor_tensor(
                    bb[:], mu, -1.0, rr[:], ALU.mult, ALU.mult)
                sab.append((rr, bb))

            for i in range(2):
                ob = obs[i]
                for j in range(nj):
                    rr, bb = sab[j % 2]
                    g = j // 2
                    eng = APPLY_ROUTE[j % len(APPLY_ROUTE)]
                    if eng == "A":
                        nc.scalar.activation(
                            ob[:, j, :], ob[:, j, :], AF.Identity,
                            bias=bb[:, i, g:g + 1], scale=rr[:, i, g:g + 1])
                    else:
                        e = nc.vector if eng == "V" else nc.gpsimd
                        e.tensor_scalar(
                            ob[:, j, :], ob[:, j, :],
                            rr[:, i, g:g + 1], bb[:, i, g:g + 1],
                            ALU.mult, ALU.add)
                nc.sync.dma_start(
                    out_d[2 * p + i].rearrange("(k j) d -> k j d", k=128),
                    ob[:])
    nc.compile()
    return nc


def _host_prep(x, W, Wm, time_mask, sensor_mask, n_cores):
    """Shard along batch; transpose/augment per-core inputs (bf16)."""
    b, t_len, c = x.shape
    d = W.shape[1]
    bpc = b // n_cores
    npair = bpc // 2
    nj = t_len // MTILE

    tm = np.ascontiguousarray(time_mask).astype(np.float32)
    sm = np.ascontiguousarray(sensor_mask).astype(np.float32)
    x = np.asarray(x, dtype=np.float32)
    W = np.asarray(W, dtype=np.float32)
    Wm = np.asarray(Wm, dtype=np.float32)

    xm = x * (1.0 - tm)[:, :, None]
    # pair-packed 128 partitions: batch A rows 0..52, batch B rows 64..116
    xaug = np.zeros((b // 2, 128, t_len), np.float32)
    xpairs = xm.reshape(b // 2, 2, t_len, c)
    tmp = tm.reshape(b // 2, 2, t_len)
    for half in range(2):
        rb = 64 * half
        xaug[:, rb:rb + c] = xpairs[:, half].transpose(0, 2, 1)
        xaug[:, rb + c] = 1.0
        xaug[:, rb + c + 1] = tmp[:, half]
    # free layout (j, m): token t = m*nj + j -> chunk j contiguous [*, 128]
    nj = t_len // MTILE
    xaug = (xaug.reshape(b // 2, 128, MTILE, nj).transpose(0, 1, 3, 2)
            .reshape(b // 2, 128, t_len))
    xaug = xaug.astype(ml_dtypes.bfloat16)

    allWm = Wm.sum(axis=0)
    smWm = sm @ Wm
    waug_b = np.empty((b, CAUG, d), np.float32)
    waug_b[:, :c] = W[None] * (1.0 - sm)[:, :, None]
    waug_b[:, c] = smWm
    waug_b[:, c + 1] = allWm[None] - smWm
    waug = np.zeros((b // 2, 128, d), np.float32)
    wpairs = waug_b.reshape(b // 2, 2, CAUG, d)
    waug[:, 0:CAUG] = wpairs[:, 0]
    waug[:, 64:64 + CAUG] = wpairs[:, 1]
    waug = waug.astype(ml_dtypes.bfloat16)

    in_maps = []
    for m in range(n_cores):
        sl = slice(m * npair, (m + 1) * npair)
        in_maps.append({
            "xaug": np.ascontiguousarray(xaug[sl]),
            "waug": np.ascontiguousarray(waug[sl]),
        })
    return in_maps


_NC_CACHE = {}


def kernel(x, W, Wm, gamma, beta, time_mask, sensor_mask):
    x = np.asarray(x)
    b, t_len, c = x.shape
    n_cores = N_CORES
    bpc = b // n_cores
    npair = bpc // 2

    key = (npair, t_len)
    if key not in _NC_CACHE:
        _NC_CACHE[key] = build_nc(npair, t_len)
    nc = _NC_CACHE[key]

    in_maps = _host_prep(x, W, Wm, time_mask, sensor_mask, n_cores)

    trace = bool(int(os.environ.get("KERNEL_TRACE", "0")))
    res = run_bass_kernel_spmd(nc, in_maps, list(range(n_cores)), trace=trace)
    kernel.last_results = res

    out = np.concatenate(
        [np.asarray(res.results[i]["out"]) for i in range(n_cores)], axis=0)

    out = out.astype(np.float32)
    gamma = np.asarray(gamma, dtype=np.float32)
    beta = np.asarray(beta, dtype=np.float32)
    if not (np.all(gamma == 1.0) and np.all(beta == 0.0)):
        out = out * gamma + beta
    return out


# revision 19
# speedup vs baseline: 2.4441x; 1.0154x over previous
"""Trainium2 Bass kernel for InputProjection + time/sensor masking + LayerNorm.

Reference computation (B=64, T=4096, C=51, D=64):
    mask[b,t,c] = time_mask[b,t] | sensor_mask[b,c]
    out = LN( einsum('btc,cd->btd', x*(1-mask), W) + einsum('btc,cd->btd', mask, Wm) )

Algebraic restructure (exact):
    With W_b[c,d]   = (1 - sm[b,c]) * W[c,d]
         smWm_b[d]  = sum_c sm[b,c]*Wm[c,d]
         allWm[d]   = sum_c Wm[c,d]
    pre[b,t,d] = sum_c x[b,t,c]*(1-tm[b,t]) * W_b[c,d]
               + 1 * smWm_b[d]
               + tm[b,t] * (allWm - smWm_b)[d]
    (for tm=1 rows the x-term vanishes and pre = allWm exactly, whose LN equals the
     reference's masked-row output; no select needed.)

Device kernel v2 (per core, data-parallel over batch; all I/O bf16):
    - augmented transposed inputs, two batches packed per 128 partitions:
        xaug[pair, half, 53, nj, 128]: rows 0..50 = (x*(1-tm)).T, row 51 = 1,
            row 52 = tm; chunk j holds tokens t = m*nj + j (m = psum partition)
            and is contiguous [53,128] for fast weight load.
        waug[pair, half, 53, D]: rows 0..50 = W_b, 51 = smWm_b, 52 = allWm-smWm_b.
    - per 128-token chunk: one 53-deep bf16 matmul -> PSUM [128t, 64d] fp32
    - per 2 PSUM banks (16 chunks): one wide ACT Copy evict -> SBUF bf16
      (amortizes the ~352-cycle ACT fixed overhead)
    - per chunk pair: one bn_stats whose INPUT AP interleaves the two chunks
      element-wise ([128, 64d, 2]) so the even/odd triple split lands on
      chunk A/B exactly -- full per-chunk stats, no combine chain, and the
      matmul PSUM writes stay contiguous. (The BIR verifier requires 6
      stats/partition per bn_stats, so multi-group 3D bn_stats is out.)
    - per pair: short s/b chain (rsqrt via ACT Sqrt + DVE reciprocal;
      tolerance 2e-2 so no Newton step)
    - per chunk: in-place DVE tensor_scalar (mult,add) apply on the bf16 SBUF
      copy (4x perf mode) then DMA out (each partition writes one contiguous
      nj*64*2B block).
    gamma/beta applied on host only if nontrivial (reference uses 1/0).
"""

import os
import sys
from contextlib import ExitStack

import numpy as np
import ml_dtypes

for _p in ("/opt/trn_rl_repo", "/root/.axon_site/_ro/trn_rl_repo"):
    if os.path.isdir(_p) and _p not in sys.path:
        sys.path.insert(0, _p)

import concourse.bass as bass
import concourse.bacc as bacc
import concourse.mybir as mybir
from concourse import tile
from concourse.bass_utils import run_bass_kernel_spmd

F32 = mybir.dt.float32
BF16 = mybir.dt.bfloat16
AF = mybir.ActivationFunctionType
ALU = mybir.AluOpType

B, T, C, D = 64, 4096, 51, 64
LN_EPS = 1e-5
N_CORES = 8
BPC = B // N_CORES          # batches per core
NPAIR = BPC // 2            # batch pairs per core
CAUG = C + 2                # augmented contraction depth (x rows + ones + tm)
MTILE = 128                 # tokens per matmul chunk (psum partitions)
BANK = 8                    # chunks per PSUM bank (8*64 fp32 = 512 = one bank)
# per-chunk LN-apply engine routing (V=DVE ~203ns, A=ACT ~347ns, G=GPSIMD ?):
# DVE also carries bn_stats (27us), ACT the wide evicts (22us), GPSIMD is idle
APPLY_ROUTE = tuple(os.environ.get("KERNEL_APPLY_ROUTE", "VAGGVAGGVAGGVAGG"))


def _bn_stats_stream(nc, out_ap, in_ap):
    """bn_stats with a multi-dim input AP treated as ONE positional stream.

    The HW's even/odd triple split is by stream position (dual accumulator
    pipes), so a [128, d, 2] interleaving AP yields chunk-A stats in the even
    triple and chunk-B in the odd one. bass's bn_stats wrapper would treat the
    extra AP dim as a stats "group" and demand a 6*G output (which the BIR
    verifier rejects anyway); emit the raw instruction instead.
    """
    eng = nc.vector
    return eng.add_instruction(
        mybir.InstBNStats(
            name=eng.bass.get_next_instruction_name(),
            ins=[eng.lower_ap(in_ap)],
            outs=[eng.lower_ap(out_ap)],
        )
    )


def build_nc(npair: int, t_len: int, debug: bool = False):
    """Build the per-core Bass program. Identical on all cores (SPMD)."""
    nj = t_len // MTILE                 # chunks per batch
    assert t_len % (MTILE * BANK) == 0, "t_len must be a multiple of 1024"
    nbank = nj // BANK                  # psum banks per batch

    nc = bacc.Bacc("TRN2", target_bir_lowering=False, debug=debug)
    # full 128-partition, flat-2D DMA shapes: partial-partition / 3D-AP
    # transfers fall off the distributed DGE path onto a single serialized
    # queue (measured: 930 descriptors on DMA_0 + 5.5us DIRECT2D per
    # dma_start on Sync). The 22 zero partitions cost ~20% extra bytes but
    # keep all 16 DMA engines fed.
    xaug_d = nc.dram_tensor("xaug", [npair, 128, t_len], BF16,
                            kind="ExternalInput")
    waug_d = nc.dram_tensor("waug", [npair, 128, D], BF16,
                            kind="ExternalInput")
    out_d = nc.dram_tensor("out", [2 * npair, t_len, D], BF16,
                           kind="ExternalOutput")

    with tile.TileContext(nc) as tc, ExitStack() as ctx:
        wpool = ctx.enter_context(tc.tile_pool(name="wpool", bufs=1))
        xpool = ctx.enter_context(tc.tile_pool(name="xpool", bufs=3))
        opool = ctx.enter_context(tc.tile_pool(name="opool", bufs=5))
        spool = ctx.enter_context(tc.tile_pool(name="spool", bufs=3))
        tpool = ctx.enter_context(tc.tile_pool(name="tpool", bufs=3))
        psum = ctx.enter_context(tc.tile_pool(name="psum", bufs=4, space="PSUM"))

        wa = wpool.tile([128, npair, D], BF16)
        nc.sync.dma_start(wa[:], waug_d.rearrange("n k d -> k n d"))

        for p in range(npair):
            xat = xpool.tile([128, t_len], BF16)
            nc.sync.dma_start(xat[:], xaug_d[p])
            # chunk j = contiguous 128-col block (token t = m*nj + j)
            xa = xat[:].rearrange("k (j m) -> k j m", m=MTILE)

            # stats triples per chunk pair: slots 0..2 = even chunk (2q),
            # slots 3..5 = odd chunk (2q+1); count = 64 each.
            stp = spool.tile([128, 2, nj // 2, 6], F32)
            obs = []
            for i in range(2):
                rb = 64 * i
                ob = opool.tile([128, nj, D], BF16, tag="ob")
                obs.append(ob)
                for h in range(nbank // 2):
                    ps = psum.tile([128, 2, BANK, D], F32, tag="psbank")
                    for hb in range(2):
                        for q in range(BANK):
                            j = (2 * h + hb) * BANK + q
                            nc.tensor.matmul(
                                ps[:, hb, q, :],
                                xa[rb:rb + CAUG, j, :],
                                wa[rb:rb + CAUG, p, :],
                                start=True,
                                stop=True,
                            )
                        # interleave-AP bn_stats per chunk pair: stream
                        # A0,B0,A1,B1,... so even/odd triples = chunk A/B
                        for q in range(BANK // 2):
                            g = (2 * h + hb) * (BANK // 2) + q
                            _bn_stats_stream(
                                nc, stp[:, i, g, :],
                                ps[:, hb, 2 * q:2 * q + 2, :].rearrange(
                                    "p a d -> p d a"))
                    # wide evict: two banks -> bf16 SBUF in one ACT op
                    nc.scalar.activation(
                        ob[:, 2 * h * BANK:2 * (h + 1) * BANK, :], ps[:],
                        AF.Copy)

            # s = rsqrt(var+eps), b = -mu*s per chunk; run the short chain
            # once over the even-chunk slots and once over the odd-chunk
            # slots ([128, 2*nj/2] strided views, count=64 -> var = M2/64).
            sab = []
            for mf, cf in ((1, 2), (4, 5)):
                mu = stp[:, :, :, mf]
                M2 = stp[:, :, :, cf]
                veps = tpool.tile([128, 2, nj // 2], F32, tag=f"veps{mf}")
                sq = tpool.tile([128, 2, nj // 2], F32, tag=f"sq{mf}")
                rr = tpool.tile([128, 2, nj // 2], F32, tag=f"rr{mf}")
                bb = tpool.tile([128, 2, nj // 2], F32, tag=f"bb{mf}")
                nc.vector.tensor_scalar(veps[:], M2, 1.0 / D, LN_EPS,
                                        ALU.mult, ALU.add)
                nc.scalar.activation(sq[:], veps[:], AF.Sqrt)
                nc.vector.reciprocal(rr[:], sq[:])
                nc.vector.scalar_tensor_tensor(
                    bb[:], mu, -1.0, rr[:], ALU.mult, ALU.mult)
                sab.append((rr, bb))

            for i in range(2):
                ob = obs[i]
                for j in range(nj):
                    rr, bb = sab[j % 2]
                    g = j // 2
                    eng = APPLY_ROUTE[j % len(APPLY_ROUTE)]
                    if eng == "A":
                        nc.scalar.activation(
                            ob[:, j, :], ob[:, j, :], AF.Identity,
                            bias=bb[:, i, g:g + 1], scale=rr[:, i, g:g + 1])
                    else:
                        e = nc.vector if eng == "V" else nc.gpsimd
                        e.tensor_scalar(
                            ob[:, j, :], ob[:, j, :],
                            rr[:, i, g:g + 1], bb[:, i, g:g + 1],
                            ALU.mult, ALU.add)
                nc.sync.dma_start(
                    out_d[2 * p + i].rearrange("(k j) d -> k j d", k=128),
                    ob[:])
    nc.compile()
    return nc


def _host_prep(x, W, Wm, time_mask, sensor_mask, n_cores):
    """Shard along batch; transpose/augment per-core inputs (bf16)."""
    b, t_len, c = x.shape
    d = W.shape[1]
    bpc = b // n_cores
    npair = bpc // 2
    nj = t_len // MTILE

    tm = np.ascontiguousarray(time_mask).astype(np.float32)
    sm = np.ascontiguousarray(sensor_mask).astype(np.float32)
    x = np.asarray(x, dtype=np.float32)
    W = np.asarray(W, dtype=np.float32)
    Wm = np.asarray(Wm, dtype=np.float32)

    xm = x * (1.0 - tm)[:, :, None]
    # pair-packed 128 partitions: batch A rows 0..52, batch B rows 64..116
    xaug = np.zeros((b // 2, 128, t_len), np.float32)
    xpairs = xm.reshape(b // 2, 2, t_len, c)
    tmp = tm.reshape(b // 2, 2, t_len)
    for half in range(2):
        rb = 64 * half
        xaug[:, rb:rb + c] = xpairs[:, half].transpose(0, 2, 1)
        xaug[:, rb + c] = 1.0
        xaug[:, rb + c + 1] = tmp[:, half]
    # free layout (j, m): token t = m*nj + j -> chunk j contiguous [*, 128]
    nj = t_len // MTILE
    xaug = (xaug.reshape(b // 2, 128, MTILE, nj).transpose(0, 1, 3, 2)
            .reshape(b // 2, 128, t_len))
    xaug = xaug.astype(ml_dtypes.bfloat16)

    allWm = Wm.sum(axis=0)
    smWm = sm @ Wm
    waug_b = np.empty((b, CAUG, d), np.float32)
    waug_b[:, :c] = W[None] * (1.0 - sm)[:, :, None]
    waug_b[:, c] = smWm
    waug_b[:, c + 1] = allWm[None] - smWm
    waug = np.zeros((b // 2, 128, d), np.float32)
    wpairs = waug_b.reshape(b // 2, 2, CAUG, d)
    waug[:, 0:CAUG] = wpairs[:, 0]
    waug[:, 64:64 + CAUG] = wpairs[:, 1]
    waug = waug.astype(ml_dtypes.bfloat16)

    in_maps = []
    for m in range(n_cores):
        sl = slice(m * npair, (m + 1) * npair)
        in_maps.append({
            "xaug": np.ascontiguousarray(xaug[sl]),
            "waug": np.ascontiguousarray(waug[sl]),
        })
    return in_maps


_NC_CACHE = {}


def kernel(x, W, Wm, gamma, beta, time_mask, sensor_mask):
    x = np.asarray(x)
    b, t_len, c = x.shape
    n_cores = N_CORES
    bpc = b // n_cores
    npair = bpc // 2

    key = (npair, t_len)
    if key not in _NC_CACHE:
        _NC_CACHE[key] = build_nc(npair, t_len)
    nc = _NC_CACHE[key]

    in_maps = _host_prep(x, W, Wm, time_mask, sensor_mask, n_cores)

    trace = bool(int(os.environ.get("KERNEL_TRACE", "0")))
    res = run_bass_kernel_spmd(nc, in_maps, list(range(n_cores)), trace=trace)
    kernel.last_results = res

    out = np.concatenate(
        [np.asarray(res.results[i]["out"]) for i in range(n_cores)], axis=0)

    out = out.astype(np.float32)
    gamma = np.asarray(gamma, dtype=np.float32)
    beta = np.asarray(beta, dtype=np.float32)
    if not (np.all(gamma == 1.0) and np.all(beta == 0.0)):
        out = out * gamma + beta
    return out


# revision 20
# speedup vs baseline: 2.5371x; 1.0380x over previous
"""Trainium2 Bass kernel for InputProjection + time/sensor masking + LayerNorm.

Reference computation (B=64, T=4096, C=51, D=64):
    mask[b,t,c] = time_mask[b,t] | sensor_mask[b,c]
    out = LN( einsum('btc,cd->btd', x*(1-mask), W) + einsum('btc,cd->btd', mask, Wm) )

Algebraic restructure (exact):
    With W_b[c,d]   = (1 - sm[b,c]) * W[c,d]
         smWm_b[d]  = sum_c sm[b,c]*Wm[c,d]
         allWm[d]   = sum_c Wm[c,d]
    pre[b,t,d] = sum_c x[b,t,c]*(1-tm[b,t]) * W_b[c,d]
               + 1 * smWm_b[d]
               + tm[b,t] * (allWm - smWm_b)[d]
    (for tm=1 rows the x-term vanishes and pre = allWm exactly, whose LN equals the
     reference's masked-row output; no select needed.)

Device kernel v2 (per core, data-parallel over batch; all I/O bf16):
    - augmented transposed inputs, two batches packed per 128 partitions:
        xaug[pair, half, 53, nj, 128]: rows 0..50 = (x*(1-tm)).T, row 51 = 1,
            row 52 = tm; chunk j holds tokens t = m*nj + j (m = psum partition)
            and is contiguous [53,128] for fast weight load.
        waug[pair, half, 53, D]: rows 0..50 = W_b, 51 = smWm_b, 52 = allWm-smWm_b.
    - per 128-token chunk: one 53-deep bf16 matmul -> PSUM [128t, 64d] fp32
    - per 2 PSUM banks (16 chunks): one wide ACT Copy evict -> SBUF bf16
      (amortizes the ~352-cycle ACT fixed overhead)
    - per chunk pair: one bn_stats whose INPUT AP interleaves the two chunks
      element-wise ([128, 64d, 2]) so the even/odd triple split lands on
      chunk A/B exactly -- full per-chunk stats, no combine chain, and the
      matmul PSUM writes stay contiguous. (The BIR verifier requires 6
      stats/partition per bn_stats, so multi-group 3D bn_stats is out.)
    - per pair: short s/b chain (rsqrt via ACT Sqrt + DVE reciprocal;
      tolerance 2e-2 so no Newton step)
    - per chunk: in-place DVE tensor_scalar (mult,add) apply on the bf16 SBUF
      copy (4x perf mode) then DMA out (each partition writes one contiguous
      nj*64*2B block).
    gamma/beta applied on host only if nontrivial (reference uses 1/0).
"""

import os
import sys
from contextlib import ExitStack

import numpy as np
import ml_dtypes

for _p in ("/opt/trn_rl_repo", "/root/.axon_site/_ro/trn_rl_repo"):
    if os.path.isdir(_p) and _p not in sys.path:
        sys.path.insert(0, _p)

import concourse.bass as bass
import concourse.bacc as bacc
import concourse.mybir as mybir
from concourse import tile
from concourse.bass_utils import run_bass_kernel_spmd

F32 = mybir.dt.float32
BF16 = mybir.dt.bfloat16
AF = mybir.ActivationFunctionType
ALU = mybir.AluOpType

B, T, C, D = 64, 4096, 51, 64
LN_EPS = 1e-5
N_CORES = 8
BPC = B // N_CORES          # batches per core
NPAIR = BPC // 2            # batch pairs per core
CAUG = C + 2                # augmented contraction depth (x rows + ones + tm)
MTILE = 128                 # tokens per matmul chunk (psum partitions)
BANK = 8                    # chunks per PSUM bank (8*64 fp32 = 512 = one bank)
# per-chunk LN-apply engine routing (V=DVE ~203ns, A=ACT ~347ns, G=GPSIMD ?):
# DVE also carries bn_stats (27us), ACT the wide evicts (22us), GPSIMD is idle
APPLY_ROUTE = tuple(os.environ.get("KERNEL_APPLY_ROUTE", "VAGGVAGGVAGGVAGG"))


def _bn_stats_stream(nc, out_ap, in_ap):
    """bn_stats with a multi-dim input AP treated as ONE positional stream.

    The HW's even/odd triple split is by stream position (dual accumulator
    pipes), so a [128, d, 2] interleaving AP yields chunk-A stats in the even
    triple and chunk-B in the odd one. bass's bn_stats wrapper would treat the
    extra AP dim as a stats "group" and demand a 6*G output (which the BIR
    verifier rejects anyway); emit the raw instruction instead.
    """
    eng = nc.vector
    return eng.add_instruction(
        mybir.InstBNStats(
            name=eng.bass.get_next_instruction_name(),
            ins=[eng.lower_ap(in_ap)],
            outs=[eng.lower_ap(out_ap)],
        )
    )


def build_nc(npair: int, t_len: int, debug: bool = False):
    """Build the per-core Bass program. Identical on all cores (SPMD)."""
    nj = t_len // MTILE                 # chunks per batch
    assert t_len % (MTILE * BANK) == 0, "t_len must be a multiple of 1024"
    nbank = nj // BANK                  # psum banks per batch

    nc = bacc.Bacc("TRN2", target_bir_lowering=False, debug=debug)
    # full 128-partition, flat-2D DMA shapes: partial-partition / 3D-AP
    # transfers fall off the distributed DGE path onto a single serialized
    # queue (measured: 930 descriptors on DMA_0 + 5.5us DIRECT2D per
    # dma_start on Sync). The 22 zero partitions cost ~20% extra bytes but
    # keep all 16 DMA engines fed.
    xaug_d = nc.dram_tensor("xaug", [npair, 128, t_len], BF16,
                            kind="ExternalInput")
    waug_d = nc.dram_tensor("waug", [npair, 128, D], BF16,
                            kind="ExternalInput")
    out_d = nc.dram_tensor("out", [2 * npair, t_len, D], BF16,
                           kind="ExternalOutput")

    with tile.TileContext(nc) as tc, ExitStack() as ctx:
        wpool = ctx.enter_context(tc.tile_pool(name="wpool", bufs=1))
        xpool = ctx.enter_context(tc.tile_pool(name="xpool", bufs=3))
        opool = ctx.enter_context(tc.tile_pool(name="opool", bufs=5))
        spool = ctx.enter_context(tc.tile_pool(name="spool", bufs=3))
        tpool = ctx.enter_context(tc.tile_pool(name="tpool", bufs=3))
        psum = ctx.enter_context(tc.tile_pool(name="psum", bufs=4, space="PSUM"))

        wa = wpool.tile([128, npair, D], BF16)
        nc.sync.dma_start(wa[:], waug_d.rearrange("n k d -> k n d"))

        for p in range(npair):
            xat = xpool.tile([128, t_len], BF16)
            # quarter-granular input DMA: matmuls on unit h wait only for
            # their own 16-chunk column slice, not the whole 1MB pair load
            nun = nj // (2 * BANK)      # 2-bank units per batch
            for u in range(nun):
                cs = slice(u * 2 * BANK * MTILE, (u + 1) * 2 * BANK * MTILE)
                nc.sync.dma_start(xat[:, cs], xaug_d[p, :, cs])
            # chunk j = contiguous 128-col block (token t = m*nj + j)
            xa = xat[:].rearrange("k (j m) -> k j m", m=MTILE)

            for i in range(2):
                rb = 64 * i
                # stats triples per chunk pair: slots 0..2 = even chunk,
                # slots 3..5 = odd chunk; count = 64 each.
                stp = spool.tile([128, nj // 2, 6], F32, tag="stp")
                ob = opool.tile([128, nj, D], F32, tag="ob")
                fin = opool.tile([128, nj, D], BF16, tag="fin")
                for h in range(nun):
                    ps = psum.tile([128, 2, BANK, D], F32, tag="psbank")
                    for hb in range(2):
                        for q in range(BANK):
                            j = (2 * h + hb) * BANK + q
                            nc.tensor.matmul(
                                ps[:, hb, q, :],
                                xa[rb:rb + CAUG, j, :],
                                wa[rb:rb + CAUG, p, :],
                                start=True,
                                stop=True,
                            )
                        # interleave-AP bn_stats per chunk pair: stream
                        # A0,B0,A1,B1,... so even/odd triples = chunk A/B
                        for q in range(BANK // 2):
                            g = (2 * h + hb) * (BANK // 2) + q
                            _bn_stats_stream(
                                nc, stp[:, g, :],
                                ps[:, hb, 2 * q:2 * q + 2, :].rearrange(
                                    "p a d -> p d a"))
                    # wide evict: two banks -> fp32 SBUF in one ACT op
                    # (fp32 keeps the DVE applies eligible for 2x_2P mode)
                    nc.scalar.activation(
                        ob[:, 2 * h * BANK:2 * (h + 1) * BANK, :], ps[:],
                        AF.Copy)

                # s = rsqrt(var+eps), b = -mu*s per chunk; short chain over
                # even-chunk slots then odd-chunk slots (no Newton step:
                # tolerance is 2e-2).
                sab = []
                for mf, cf in ((1, 2), (4, 5)):
                    mu = stp[:, :, mf]
                    M2 = stp[:, :, cf]
                    veps = tpool.tile([128, nj // 2], F32, tag=f"veps{mf}")
                    sq = tpool.tile([128, nj // 2], F32, tag=f"sq{mf}")
                    rr = tpool.tile([128, nj // 2], F32, tag=f"rr{mf}")
                    bb = tpool.tile([128, nj // 2], F32, tag=f"bb{mf}")
                    nc.vector.tensor_scalar(veps[:], M2, 1.0 / D, LN_EPS,
                                            ALU.mult, ALU.add)
                    nc.scalar.activation(sq[:], veps[:], AF.Sqrt)
                    nc.vector.reciprocal(rr[:], sq[:])
                    nc.vector.scalar_tensor_tensor(
                        bb[:], mu, -1.0, rr[:], ALU.mult, ALU.mult)
                    sab.append((rr, bb))

                for u in range(2):
                    for j in range(u * nj // 2, (u + 1) * nj // 2):
                        rr, bb = sab[j % 2]
                        g = j // 2
                        eng = APPLY_ROUTE[j % len(APPLY_ROUTE)]
                        if eng == "A":
                            nc.scalar.activation(
                                fin[:, j, :], ob[:, j, :], AF.Identity,
                                bias=bb[:, g:g + 1], scale=rr[:, g:g + 1])
                        else:
                            e = nc.vector if eng == "V" else nc.gpsimd
                            e.tensor_scalar(
                                fin[:, j, :], ob[:, j, :],
                                rr[:, g:g + 1], bb[:, g:g + 1],
                                ALU.mult, ALU.add)
                    # half-batch output DMA overlaps the remaining applies
                    half = slice(u * nj // 2, (u + 1) * nj // 2)
                    nc.sync.dma_start(
                        out_d[2 * p + i].rearrange(
                            "(k j) d -> k j d", k=128)[:, half],
                        fin[:, half])
    nc.compile()
    return nc


def _host_prep(x, W, Wm, time_mask, sensor_mask, n_cores):
    """Shard along batch; transpose/augment per-core inputs (bf16)."""
    b, t_len, c = x.shape
    d = W.shape[1]
    bpc = b // n_cores
    npair = bpc // 2
    nj = t_len // MTILE

    tm = np.ascontiguousarray(time_mask).astype(np.float32)
    sm = np.ascontiguousarray(sensor_mask).astype(np.float32)
    x = np.asarray(x, dtype=np.float32)
    W = np.asarray(W, dtype=np.float32)
    Wm = np.asarray(Wm, dtype=np.float32)

    xm = x * (1.0 - tm)[:, :, None]
    # pair-packed 128 partitions: batch A rows 0..52, batch B rows 64..116
    xaug = np.zeros((b // 2, 128, t_len), np.float32)
    xpairs = xm.reshape(b // 2, 2, t_len, c)
    tmp = tm.reshape(b // 2, 2, t_len)
    for half in range(2):
        rb = 64 * half
        xaug[:, rb:rb + c] = xpairs[:, half].transpose(0, 2, 1)
        xaug[:, rb + c] = 1.0
        xaug[:, rb + c + 1] = tmp[:, half]
    # free layout (j, m): token t = m*nj + j -> chunk j contiguous [*, 128]
    nj = t_len // MTILE
    xaug = (xaug.reshape(b // 2, 128, MTILE, nj).transpose(0, 1, 3, 2)
            .reshape(b // 2, 128, t_len))
    xaug = xaug.astype(ml_dtypes.bfloat16)

    allWm = Wm.sum(axis=0)
    smWm = sm @ Wm
    waug_b = np.empty((b, CAUG, d), np.float32)
    waug_b[:, :c] = W[None] * (1.0 - sm)[:, :, None]
    waug_b[:, c] = smWm
    waug_b[:, c + 1] = allWm[None] - smWm
    waug = np.zeros((b // 2, 128, d), np.float32)
    wpairs = waug_b.reshape(b // 2, 2, CAUG, d)
    waug[:, 0:CAUG] = wpairs[:, 0]
    waug[:, 64:64 + CAUG] = wpairs[:, 1]
    waug = waug.astype(ml_dtypes.bfloat16)

    in_maps = []
    for m in range(n_cores):
        sl = slice(m * npair, (m + 1) * npair)
        in_maps.append({
            "xaug": np.ascontiguousarray(xaug[sl]),
            "waug": np.ascontiguousarray(waug[sl]),
        })
    return in_maps


_NC_CACHE = {}


def kernel(x, W, Wm, gamma, beta, time_mask, sensor_mask):
    x = np.asarray(x)
    b, t_len, c = x.shape
    n_cores = N_CORES
    bpc = b // n_cores
    npair = bpc // 2

    key = (npair, t_len)
    if key not in _NC_CACHE:
        _NC_CACHE[key] = build_nc(npair, t_len)
    nc = _NC_CACHE[key]

    in_maps = _host_prep(x, W, Wm, time_mask, sensor_mask, n_cores)

    trace = bool(int(os.environ.get("KERNEL_TRACE", "0")))
    res = run_bass_kernel_spmd(nc, in_maps, list(range(n_cores)), trace=trace)
    kernel.last_results = res

    out = np.concatenate(
        [np.asarray(res.results[i]["out"]) for i in range(n_cores)], axis=0)

    out = out.astype(np.float32)
    gamma = np.asarray(gamma, dtype=np.float32)
    beta = np.asarray(beta, dtype=np.float32)
    if not (np.all(gamma == 1.0) and np.all(beta == 0.0)):
        out = out * gamma + beta
    return out
